# revision 7
# baseline (speedup 1.0000x reference)
"""GATv2 convolution on 8 Trainium2 NeuronCores (Bass/Tile).

Strategy (edge-parallel by target-node range):
  - Host: shard edges by tgt//NSLICE so each core owns all edges of its
    node slice; sort by (tile, src>=32768), pad each 128-node tile's edge
    list to uniform slot counts so one SPMD program fits all cores.
  - Device phase A: T1 = x @ w1 (full table, replicated compute) and
    T2s = x_slice @ w2 (own slice only), f32 tables in DRAM.
  - Device phase B (per 128-node tile): batch-gather T1[src]/T2s[tgt_loc]
    rows via gpsimd.dma_gather (int16 indices; src split lo/hi around
    32768), z = ti + tj, leaky_relu via one DVE scalar_tensor_tensor,
    e = z' * a, per-head sums via strided DVE reduce, w = exp(logits) on
    ACT. Segment softmax-weighted scatter-sum = one-hot matmuls
    accumulated in PSUM (numerator || denominator in one [128,136] tile),
    then per-node normalize num/den. No collectives needed.
"""

import sys

sys.path.insert(0, "/opt/trn_rl_repo")

import numpy as np
import ml_dtypes

import concourse.bass as bass
import concourse.bacc as bacc
import concourse.mybir as mybir
import concourse.tile as tile
from concourse import bass_utils
from concourse.masks import make_identity

P = 128
CORES = 8
HALF = 32768
ALPHA = 0.3
NH = 8
OC = 16

f32 = mybir.dt.float32
bf16 = mybir.dt.bfloat16
i16 = mybir.dt.int16

_last_results = None  # test harness reads exec_time_ns from here


def _roundup(v, m):
    return (v + m - 1) // m * m


def _wrap16(arr):
    """[..., n] int -> [..., 128, n//16] int16 in dma_gather's wrapped layout:
    index i lives at partition i%16, slot i//16, replicated to 128 partitions."""
    *lead, n = arr.shape
    w = arr.reshape(*lead, n // 16, 16)
    w = np.swapaxes(w, -1, -2)  # [..., 16, n//16]
    w = np.tile(w, (*([1] * len(lead)), 8, 1))  # [..., 128, n//16]
    return np.ascontiguousarray(w.astype(np.int16))


def _host_prep(x, w1, w2, a, src, tgt):
    N, CH = x.shape
    E = src.shape[0]
    assert CH == 128 and N % CORES == 0
    nslice = N // CORES
    nt_b = _roundup(nslice, P) // P

    src = src.astype(np.int64)
    tgt = tgt.astype(np.int64)
    core = tgt // nslice
    tloc = tgt - core * nslice
    tile_i = tloc // P
    loc = tloc % P
    is_hi = (src >= HALF).astype(np.int64)

    ngroups = CORES * nt_b * 2
    key = (core * nt_b + tile_i) * 2 + is_hi
    order = np.argsort(key, kind="stable")
    key_s = key[order]
    src_s = src[order]
    tloc_s = tloc[order]
    loc_s = loc[order]
    core_s = core[order]
    tile_s = tile_i[order]
    hi_s = is_hi[order]

    counts = np.bincount(key, minlength=ngroups).reshape(CORES, nt_b, 2)
    n_lo = counts[:, :, 0]
    n_hi = counts[:, :, 1]
    s_lo = int(_roundup(max(int(n_lo.max()), 16), P))
    s_hi = int(_roundup(max(int(n_hi.max()), 16), P))
    ts = s_lo + s_hi

    gstart = np.zeros(ngroups, dtype=np.int64)
    gstart[1:] = np.cumsum(counts.ravel())[:-1]
    rank = np.arange(E, dtype=np.int64) - gstart[key_s]
    slot = rank + np.where(hi_s == 1, s_lo, 0)

    src_arr = np.zeros((CORES, nt_b, ts), dtype=np.int64)
    tgl_arr = np.full((CORES, nt_b, ts), -1.0, dtype=np.float32)
    tl_arr = np.zeros((CORES, nt_b, ts), dtype=np.int64)
    src_arr[core_s, tile_s, slot] = np.where(hi_s == 1, src_s - HALF, src_s)
    tgl_arr[core_s, tile_s, slot] = loc_s.astype(np.float32)
    tl_arr[core_s, tile_s, slot] = tloc_s

    slo = _wrap16(src_arr[:, :, :s_lo])
    shi = _wrap16(src_arr[:, :, s_lo:])
    tlo = _wrap16(tl_arr)
    # tgtloc column-major per 128-edge chunk: [.., 128, T_C]
    t_c = ts // P
    tgl = np.ascontiguousarray(
        tgl_arr.reshape(CORES, nt_b, t_c, P).transpose(0, 1, 3, 2)
    ).astype(ml_dtypes.bfloat16)

    n_pad = _roundup(N, P)
    x_pad = np.zeros((n_pad, CH), dtype=np.float32)
    x_pad[:N] = x
    xs_pad = np.zeros((CORES, nt_b * P, CH), dtype=np.float32)
    for c in range(CORES):
        xs_pad[c, :nslice] = x[c * nslice : (c + 1) * nslice]
    w12 = np.concatenate([w1, w2], axis=1).astype(np.float32)
    a_bc = np.tile(a.reshape(1, CH).astype(np.float32), (P, 1))
    iota = np.tile(np.arange(P, dtype=np.float32)[None, :], (P, 1)).astype(
        ml_dtypes.bfloat16
    )

    in_maps = []
    for c in range(CORES):
        in_maps.append(
            {
                "x": x_pad,
                "xs": np.ascontiguousarray(xs_pad[c]),
                "w12": w12,
                "a_bc": a_bc,
                "iota": iota,
                "slo": np.ascontiguousarray(slo[c]),
                "shi": np.ascontiguousarray(shi[c]),
                "tlo": np.ascontiguousarray(tlo[c]),
                "tgl": np.ascontiguousarray(tgl[c]),
            }
        )
    dims = dict(
        N=N, CH=CH, nslice=nslice, nt_b=nt_b, n_pad=n_pad,
        s_lo=s_lo, s_hi=s_hi, ts=ts, t_c=t_c,
    )
    return in_maps, dims


def _build_program(dims):
    N = dims["N"]
    CH = dims["CH"]
    nslice = dims["nslice"]
    nt_b = dims["nt_b"]
    n_pad = dims["n_pad"]
    s_lo = dims["s_lo"]
    s_hi = dims["s_hi"]
    ts = dims["ts"]
    t_c = dims["t_c"]
    nt_a = n_pad // P
    xsr = nt_b * P

    nc = bacc.Bacc("TRN2", target_bir_lowering=False, debug=False,
                   num_devices=CORES)

    x_in = nc.dram_tensor("x", [n_pad, CH], f32, kind="ExternalInput")
    xs_in = nc.dram_tensor("xs", [xsr, CH], f32, kind="ExternalInput")
    w12_in = nc.dram_tensor("w12", [CH, 2 * CH], f32, kind="ExternalInput")
    abc_in = nc.dram_tensor("a_bc", [P, CH], f32, kind="ExternalInput")
    iota_in = nc.dram_tensor("iota", [P, P], bf16, kind="ExternalInput")
    slo_in = nc.dram_tensor("slo", [nt_b, P, s_lo // 16], i16, kind="ExternalInput")
    shi_in = nc.dram_tensor("shi", [nt_b, P, s_hi // 16], i16, kind="ExternalInput")
    tlo_in = nc.dram_tensor("tlo", [nt_b, P, ts // 16], i16, kind="ExternalInput")
    tgl_in = nc.dram_tensor("tgl", [nt_b, P, t_c], bf16, kind="ExternalInput")
    out = nc.dram_tensor("out", [nslice, CH], f32, kind="ExternalOutput")
    # tables must be ExternalOutput: dma_gather from Internal DRAM pool
    # tiles crashes the exec unit (bad base relocation for custom Q7 DMA)
    t1 = nc.dram_tensor("t1tab", [n_pad, CH], f32, kind="ExternalOutput")
    t2 = nc.dram_tensor("t2tab", [xsr, CH], f32, kind="ExternalOutput")

    with tile.TileContext(nc) as tc:
        with (
            tc.tile_pool(name="const", bufs=1) as cp,
        ):

            ident = cp.tile([P, P], f32)
            make_identity(nc, ident[:])
            w12t = cp.tile([CH, 2 * CH], f32)
            nc.sync.dma_start(out=w12t[:], in_=w12_in[:])
            a_t = cp.tile([P, CH], f32)
            nc.sync.dma_start(out=a_t[:], in_=abc_in[:])
            iota_t = cp.tile([P, P], bf16)
            nc.sync.dma_start(out=iota_t[:], in_=iota_in[:])

            # ---------------- Phase A: projection tables ----------------
            with (
                tc.tile_pool(name="pa", bufs=3) as pa,
                tc.tile_pool(name="pa_ps", bufs=2, space="PSUM") as pa_ps,
                tc.tile_pool(name="pa_ps2", bufs=2, space="PSUM") as pa_ps2,
            ):
                def project(src_ap, dst_ap, w_sl, t, eng):
                    xt = pa.tile([P, CH], f32, tag="xt")
                    nc.sync.dma_start(out=xt[:], in_=src_ap)
                    psT = pa_ps.tile([P, P], f32, space="PSUM", tag="psT")
                    nc.tensor.transpose(out=psT[:], in_=xt[:], identity=ident[:])
                    xT = pa.tile([P, P], f32, tag="xT")
                    if eng == 0:
                        nc.vector.tensor_copy(out=xT[:], in_=psT[:])
                    else:
                        nc.scalar.copy(out=xT[:], in_=psT[:])
                    mm = pa_ps2.tile([P, CH], f32, space="PSUM", tag="mm")
                    nc.tensor.matmul(out=mm[:], lhsT=xT[:], rhs=w_sl,
                                     start=True, stop=True)
                    o = pa.tile([P, CH], f32, tag="o")
                    if eng == 0:
                        nc.scalar.copy(out=o[:], in_=mm[:])
                    else:
                        nc.vector.tensor_copy(out=o[:], in_=mm[:])
                    nc.sync.dma_start(out=dst_ap, in_=o[:])

                for t in range(nt_a):
                    project(x_in[t * P : (t + 1) * P, :],
                            t1[t * P : (t + 1) * P, :],
                            w12t[:, 0:CH], t, t % 2)
                for t in range(nt_b):
                    project(xs_in[t * P : (t + 1) * P, :],
                            t2[t * P : (t + 1) * P, :],
                            w12t[:, CH : 2 * CH], t, t % 2)

            # ---------------- Phase B: edge processing ----------------
            import os
            kstage = os.environ.get("KSTAGE", "full")
            with (
                tc.tile_pool(name="pb", bufs=2) as pb,
                tc.tile_pool(name="pb_ps", bufs=2, space="PSUM") as pb_ps,
            ):
                for t in range(nt_b):
                    if kstage == "A":
                        zt = pb.tile([P, CH], f32, tag="zt")
                        nc.vector.tensor_copy(out=zt[:], in_=a_t[:])
                        rows = min(P, nslice - t * P)
                        nc.sync.dma_start(
                            out=out[t * P : t * P + rows, :], in_=zt[:rows, :]
                        )
                        continue
                    li = pb.tile([P, s_lo // 16], i16, tag="li")
                    nc.sync.dma_start(out=li[:], in_=slo_in[t])
                    hi = pb.tile([P, s_hi // 16], i16, tag="hi")
                    nc.sync.dma_start(out=hi[:], in_=shi_in[t])
                    tli = pb.tile([P, ts // 16], i16, tag="tli")
                    nc.sync.dma_start(out=tli[:], in_=tlo_in[t])
                    tg = pb.tile([P, t_c], bf16, tag="tg")
                    nc.sync.dma_start(out=tg[:], in_=tgl_in[t])

                    GMAX = 1024  # dma_gather crashes above 1024 idxs/call

                    def gather_split(dst, dst_off, src_ap, idx_tile, n):
                        for off in range(0, n, GMAX):
                            sz = min(GMAX, n - off)
                            o = dst_off + off
                            nc.gpsimd.dma_gather(
                                out_ap=dst[:, o // P : (o + sz) // P, :],
                                in_ap=src_ap,
                                idxs_ap=idx_tile[:, off // 16 : (off + sz) // 16],
                                num_idxs=sz,
                                num_idxs_reg=sz,
                                elem_size=CH,
                            )

                    g1 = pb.tile([P, t_c, P], f32, tag="g1")  # ti = T1[src]
                    gather_split(g1, 0, t1[:], li, s_lo)
                    if kstage == "G1":
                        zt = pb.tile([P, CH], f32, tag="zt")
                        nc.vector.tensor_copy(out=zt[:], in_=g1[:, 0, :])
                        rows = min(P, nslice - t * P)
                        nc.sync.dma_start(
                            out=out[t * P : t * P + rows, :], in_=zt[:rows, :]
                        )
                        continue
                    hi_src = t1[HALF:, :] if N > HALF else t1[:]
                    gather_split(g1, s_lo, hi_src, hi, s_hi)
                    g2 = pb.tile([P, t_c, P], f32, tag="g2")  # tj = T2s[tloc]
                    gather_split(g2, 0, t2[:], tli, ts)

                    if kstage == "G":
                        zt = pb.tile([P, CH], f32, tag="zt")
                        nc.vector.tensor_tensor(
                            out=zt[:], in0=g1[:, 0, :], in1=g2[:, 0, :],
                            op=mybir.AluOpType.add,
                        )
                        rows = min(P, nslice - t * P)
                        nc.sync.dma_start(
                            out=out[t * P : t * P + rows, :], in_=zt[:rows, :]
                        )
                        continue

                    oh = pb.tile([P, ts], bf16, tag="oh")
                    nc.vector.tensor_tensor(
                        out=oh[:].rearrange("p (k n) -> p k n", n=P),
                        in0=tg[:][:, :, None].broadcast_to([P, t_c, P]),
                        in1=iota_t[:][:, None, :].broadcast_to([P, t_c, P]),
                        op=mybir.AluOpType.is_equal,
                    )

                    g1f = g1[:].rearrange("p a b -> p (a b)")
                    g2f = g2[:].rearrange("p a b -> p (a b)")
                    z = pb.tile([P, ts], f32, tag="z")
                    nc.vector.tensor_tensor(out=z[:], in0=g1f, in1=g2f,
                                            op=mybir.AluOpType.add)
                    # leaky_relu = max(0.3*z, z); write into g2 (tj dead)
                    zp = g2f
                    nc.vector.scalar_tensor_tensor(
                        out=zp, in0=z[:], scalar=ALPHA, in1=z[:],
                        op0=mybir.AluOpType.mult, op1=mybir.AluOpType.max,
                    )
                    # e = z' * a  (a broadcast over chunks); write into z
                    ew = z[:]
                    nc.vector.tensor_tensor(
                        out=ew.rearrange("p (k c) -> p k c", c=CH),
                        in0=zp.rearrange("p (k c) -> p k c", c=CH),
                        in1=a_t[:][:, None, :].broadcast_to([P, t_c, CH]),
                        op=mybir.AluOpType.mult,
                    )
                    # logits[e, k, h] = sum_c e[k, h, c]
                    lg = pb.tile([P, t_c * NH], f32, tag="lg")
                    nc.vector.tensor_reduce(
                        out=lg[:].rearrange("p (k h) -> p k h", h=NH),
                        in_=ew.rearrange("p (k h c) -> p k h c", h=NH, c=OC),
                        axis=mybir.AxisListType.X,
                        op=mybir.AluOpType.add,
                    )
                    # scat[e, k, :] = [msg(128) | w(8)] in bf16
                    scat = pb.tile([P, t_c * 136], bf16, tag="scat")
                    scat_r = scat[:].rearrange("p (k c) -> p k c", c=136)
                    nc.scalar.activation(
                        out=scat_r[:, :, CH : CH + NH],
                        in_=lg[:].rearrange("p (k h) -> p k h", h=NH),
                        func=mybir.ActivationFunctionType.Exp,
                    )
                    w_bc = scat_r[:, :, CH : CH + NH][:, :, :, None].broadcast_to(
                        [P, t_c, NH, OC]
                    )
                    nc.vector.tensor_tensor(
                        out=scat_r[:, :, 0:CH].rearrange(
                            "p k (h c) -> p k h c", c=OC
                        ),
                        in0=g1[:].rearrange("p k (h c) -> p k h c", c=OC),
                        in1=w_bc,
                        op=mybir.AluOpType.mult,
                    )

                    acc_ps = pb_ps.tile([P, 136], f32, space="PSUM", tag="acc")
                    for k in range(t_c):
                        nc.tensor.matmul(
                            out=acc_ps[:],
                            lhsT=oh[:, k * P : (k + 1) * P],
                            rhs=scat[:, k * 136 : (k + 1) * 136],
                            start=(k == 0),
                            stop=(k == t_c - 1),
                        )

                    acc = pb.tile([P, 136], f32, tag="accs")
                    nc.scalar.copy(out=acc[:], in_=acc_ps[:])
                    dg = pb.tile([P, NH], f32, tag="dg")
                    nc.vector.tensor_scalar_max(
                        out=dg[:], in0=acc[:, CH : CH + NH], scalar1=1e-30
                    )
                    rc = pb.tile([P, NH], f32, tag="rc")
                    nc.vector.reciprocal(out=rc[:], in_=dg[:])
                    ot = pb.tile([P, CH], f32, tag="ot")
                    nc.vector.tensor_tensor(
                        out=ot[:].rearrange("p (h c) -> p h c", c=OC),
                        in0=acc[:, 0:CH].rearrange("p (h c) -> p h c", c=OC),
                        in1=rc[:][:, :, None].broadcast_to([P, NH, OC]),
                        op=mybir.AluOpType.mult,
                    )
                    rows = min(P, nslice - t * P)
                    nc.sync.dma_start(
                        out=out[t * P : t * P + rows, :], in_=ot[:rows, :]
                    )

    nc.compile()
    return nc


def kernel(x, w1, w2, a, src, tgt):
    global _last_results
    x = np.asarray(x, dtype=np.float32)
    w1 = np.asarray(w1, dtype=np.float32)
    w2 = np.asarray(w2, dtype=np.float32)
    a = np.asarray(a, dtype=np.float32)
    src = np.asarray(src)
    tgt = np.asarray(tgt)

    in_maps, dims = _host_prep(x, w1, w2, a, src, tgt)
    nc = _build_program(dims)
    import os

    trace = bool(os.environ.get("KBENCH_TRACE"))
    res = bass_utils.run_bass_kernel_spmd(
        nc, in_maps, core_ids=list(range(CORES)), trace=trace
    )
    _last_results = res
    nslice = dims["nslice"]
    out = np.empty((x.shape[0], x.shape[1]), dtype=np.float32)
    for c in range(CORES):
        out[c * nslice : (c + 1) * nslice] = res.results[c]["out"]
    return out
